# revision 7
# baseline (speedup 1.0000x reference)
"""GAT layer (single head, PyG GATConv semantics + relu) on 8 Trainium2 cores.

Strategy (destination-major):
  * Nodes are grouped into 128-node blocks, lexsorted by (deg_lo, deg_hi) so
    every block has near-uniform in-degree. Blocks are dealt round-robin to
    the 8 cores; per-slot grid shapes are equalized across cores (SPMD).
  * Each core builds a full feature table in its HBM: row r = [h bf16 x64 |
    a_src f32 | pad] where h = x@W, a_src = x@(W@att_src). Table rows are
    (node_id + 1); row 0 and row N+1 are pad rows with a_src = -1e4 so that
    padded edge slots contribute exp(...) = 0.
  * Self-loops are NOT gathered: s_ii and h_i for own nodes are computed
    locally from ownxt and folded into the numerator/denominator.
  * Per destination block, incoming-edge source rows are gathered with
    dma_gather (int16 indices => the table is split at row 32768 into a lo
    and a hi table; each block's edge slots are split into a lo column range
    and a hi column range).
  * Softmax without max-subtraction: s = max(exp(z), exp(0.2 z)) with
    z = a_src[src] + a_dst[dst]; batched exp over whole superchunks.
"""

import ml_dtypes
import numpy as np

import concourse.bass as bass
import concourse.tile as tile
from concourse import bacc, mybir
from concourse.bass_utils import run_bass_kernel_spmd

P = 128
NCORES = 8
NEG_SLOPE = 0.2
EPS = 1e-16
PAD_ASRC = -1.0e4


def _ceil_to(x, m):
    return (x + m - 1) // m * m


def _preprocess(edge_index, n_nodes, lo_rows):
    """All host-side index work: blocks, grids, gather index tiles.

    Self-loops are excluded -- they are handled locally on-device."""
    src = np.asarray(edge_index[0], dtype=np.int64)
    dst = np.asarray(edge_index[1], dtype=np.int64)
    st = src + 1  # table rows
    is_hi = st >= lo_rows

    deg = np.bincount(dst, minlength=n_nodes)
    deg_lo = np.bincount(dst[~is_hi], minlength=n_nodes)
    deg_hi = deg - deg_lo

    # node blocks: lexsort descending by (deg_lo, deg_hi)
    order = np.lexsort((deg_hi, deg_lo))[::-1].copy()
    nblk_out = _ceil_to(n_nodes, P) // P
    slots = _ceil_to(nblk_out, NCORES) // NCORES
    node_at = np.full((slots * NCORES, P), -1, dtype=np.int64)
    node_at.reshape(-1)[: n_nodes] = order
    # per-block max degrees
    nd = node_at  # [NBLKS, P]
    valid = nd >= 0
    blk_deg_lo = np.where(valid, deg_lo[np.clip(nd, 0, None)], 0).max(axis=1)
    blk_deg_hi = np.where(valid, deg_hi[np.clip(nd, 0, None)], 0).max(axis=1)
    d_lo = blk_deg_lo.reshape(slots, NCORES).max(axis=1)  # per slot
    d_hi = blk_deg_hi.reshape(slots, NCORES).max(axis=1)

    # node -> (core, slot, p)
    pos = np.full(n_nodes, -1, dtype=np.int64)
    pos[order] = np.arange(n_nodes)
    b_of = pos // P
    p_of = pos % P
    core_of = b_of % NCORES
    slot_of = b_of // NCORES

    # rank of each edge within its destination node, lo-first
    eo = np.lexsort((is_hi, dst))
    dsts = dst[eo]
    sts = st[eo]
    his = is_hi[eo]
    off = np.zeros(n_nodes + 1, dtype=np.int64)
    np.cumsum(deg, out=off[1:])
    jj = np.arange(len(eo), dtype=np.int64) - off[dsts]
    jhi = jj - deg_lo[dsts]

    col_off_lo = np.zeros(slots + 1, dtype=np.int64)
    np.cumsum(d_lo, out=col_off_lo[1:])
    col_off_hi = np.zeros(slots + 1, dtype=np.int64)
    np.cumsum(d_hi, out=col_off_hi[1:])
    tot_lo = int(col_off_lo[-1])
    tot_hi = int(col_off_hi[-1])

    padhi_loc = n_nodes + 1 - lo_rows
    glo = np.zeros((NCORES, P, tot_lo), dtype=np.int64)  # pad -> lo row 0
    ghi = np.full((NCORES, P, tot_hi), padhi_loc, dtype=np.int64)

    ek = core_of[dsts]
    ei_slot = slot_of[dsts]
    ep = p_of[dsts]
    for k in range(NCORES):
        ml = (ek == k) & ~his
        glo[k][ep[ml], col_off_lo[ei_slot[ml]] + jj[ml]] = sts[ml]
        mh = (ek == k) & his
        ghi[k][ep[mh], col_off_hi[ei_slot[mh]] + jhi[mh]] = sts[mh] - lo_rows

    return dict(
        d_lo=d_lo, d_hi=d_hi, col_off_lo=col_off_lo, col_off_hi=col_off_hi,
        glo=glo, ghi=ghi, node_at=node_at, slots=slots,
    )


def _make_superchunks(d_lo, d_hi, cmax):
    """Group consecutive slots into super-chunks with <= cmax total columns.

    The first 2 and last 3 slots go in single-slot chunks so the pipeline
    ramps up quickly and the post-last-gather drain chain is short."""
    n = len(d_lo)
    scs = []
    cur = []
    cur_c = 0
    for i in range(n):
        c = int(d_lo[i] + d_hi[i])
        single = i < 2 or i >= n - 3
        if cur and (single or cur_c + c > cmax):
            scs.append(cur)
            cur = []
            cur_c = 0
        cur.append(i)
        cur_c += c
        if single:
            scs.append(cur)
            cur = []
            cur_c = 0
    if cur:
        scs.append(cur)
    return scs


def _wrap_idx(arr):
    """dma_gather index layout: [128, n/16] int16, idx i at (i%16, i//16),
    replicated across the 8 Q7 core groups."""
    n = arr.shape[0]
    assert n % 16 == 0
    w = arr.reshape(n // 16, 16).T.astype(np.int16)  # [16, n/16]
    return np.tile(w, (8, 1))


def _build_gidx(meta, scs):
    """Concatenate per-call wrapped index tiles; record call metadata."""
    col_off_lo, col_off_hi = meta["col_off_lo"], meta["col_off_hi"]
    calls = []  # per sc: (clo, chi, off16_lo, len16_lo, off16_hi, len16_hi)
    gidx = [[] for _ in range(NCORES)]
    off16 = 0
    for sc in scs:
        i0, i1 = sc[0], sc[-1] + 1
        a0, a1 = int(col_off_lo[i0]), int(col_off_lo[i1])
        b0, b1 = int(col_off_hi[i0]), int(col_off_hi[i1])
        clo, chi = a1 - a0, b1 - b0
        lo_len16 = clo * P // 16
        hi_len16 = chi * P // 16
        for k in range(NCORES):
            lo_list = meta["glo"][k][:, a0:a1].T.ravel()
            hi_list = meta["ghi"][k][:, b0:b1].T.ravel()
            gidx[k].append(_wrap_idx(lo_list))
            gidx[k].append(_wrap_idx(hi_list))
        calls.append((clo, chi, off16, lo_len16, off16 + lo_len16, hi_len16))
        off16 += lo_len16 + hi_len16
    gidx = [np.concatenate(g, axis=1) if g else np.zeros((P, 0), np.int16)
            for g in gidx]
    return gidx, calls, off16


def _build_nc(cfg):
    trows, lo_rows = cfg["trows"], cfg["lo_rows"]
    slots, scs, calls = cfg["slots"], cfg["scs"], cfg["calls"]
    col_off_lo, col_off_hi = cfg["col_off_lo"], cfg["col_off_hi"]
    gc16 = cfg["gc16"]
    f_out = cfg["f_out"]
    nblk_tbl = trows // P
    hi_rows = trows - lo_rows
    wcols = f_out + 2  # W | w_src | w_dst

    nc = bacc.Bacc("TRN2", target_bir_lowering=False, debug=False,
                   num_devices=NCORES, num_swdge_queues=4)
    xTb = nc.dram_tensor("xTb", [P, trows], mybir.dt.bfloat16, kind="ExternalInput")
    wextb = nc.dram_tensor("wextb", [P, wcols], mybir.dt.bfloat16,
                           kind="ExternalInput")
    ownxt = nc.dram_tensor("ownxt", [P, slots * P], mybir.dt.bfloat16,
                           kind="ExternalInput")
    gidx_d = nc.dram_tensor("gidx", [P, max(gc16, 16)], mybir.dt.int16,
                            kind="ExternalInput")
    biasb = nc.dram_tensor("biasb", [P, f_out], mybir.dt.float32,
                           kind="ExternalInput")
    padrow = nc.dram_tensor("padrow", [1, P], mybir.dt.bfloat16,
                            kind="ExternalInput")
    out_d = nc.dram_tensor("out", [slots * P, f_out], mybir.dt.float32,
                           kind="ExternalOutput")
    tbl_lo = nc.dram_tensor("tbl_lo", [lo_rows, P], mybir.dt.bfloat16,
                            kind="Internal")
    tbl_hi = nc.dram_tensor("tbl_hi", [max(hi_rows, P), P], mybir.dt.bfloat16,
                            kind="Internal")

    fp32 = mybir.dt.float32
    bf16 = mybir.dt.bfloat16

    with tile.TileContext(nc) as tc:
        with (
            tc.tile_pool(name="const", bufs=1) as cpool,
            tc.tile_pool(name="xt", bufs=3) as xtpool,
            tc.tile_pool(name="ps", bufs=4, space="PSUM") as pspool,
            tc.tile_pool(name="tstage", bufs=3) as tspool,
            tc.tile_pool(name="gat", bufs=4) as gpool,
            tc.tile_pool(name="sc", bufs=2) as scpool,
            tc.tile_pool(name="blk", bufs=4) as bpool,
        ):
            wextb_sb = cpool.tile([P, wcols], bf16)
            nc.sync.dma_start(out=wextb_sb[:], in_=wextb[:])
            biasb_sb = cpool.tile([P, f_out], fp32)
            nc.sync.dma_start(out=biasb_sb[:], in_=biasb[:])
            ownxt_sb = cpool.tile([P, slots * P], bf16)
            nc.sync.dma_start(out=ownxt_sb[:], in_=ownxt[:])
            gidx_sb = cpool.tile([P, max(gc16, 16)], mybir.dt.int16)
            nc.sync.dma_start(out=gidx_sb[:], in_=gidx_d[:])
            adst_own = cpool.tile([P, slots], fp32)
            adst02 = cpool.tile([P, slots], fp32)
            asrc_own = cpool.tile([P, slots], fp32)
            s_self = cpool.tile([P, slots], fp32)
            h_own = cpool.tile([P, slots, f_out], bf16)
            epst = cpool.tile([P, slots], fp32)
            nc.vector.memset(epst[:], EPS)

            # ---- own-node quantities: a_dst, a_src, h, s_self ----
            for i in range(slots):
                ps2 = pspool.tile([P, wcols], fp32, tag="own")
                nc.tensor.matmul(out=ps2[:], lhsT=ownxt_sb[:, i * P:(i + 1) * P],
                                 rhs=wextb_sb[:], start=True, stop=True)
                if i % 2 == 0:
                    nc.scalar.copy(out=h_own[:, i, :], in_=ps2[:, 0:f_out])
                else:
                    nc.vector.tensor_copy(out=h_own[:, i, :], in_=ps2[:, 0:f_out])
                nc.vector.tensor_copy(out=asrc_own[:, i:i + 1],
                                      in_=ps2[:, f_out:f_out + 1])
                nc.vector.tensor_copy(out=adst_own[:, i:i + 1],
                                      in_=ps2[:, f_out + 1:f_out + 2])
            nc.vector.tensor_scalar_mul(adst02[:], adst_own[:], NEG_SLOPE)
            zs = cpool.tile([P, slots], fp32)
            e1s = cpool.tile([P, slots], fp32)
            nc.vector.tensor_add(zs[:], asrc_own[:], adst_own[:])
            nc.scalar.activation(out=e1s[:], in_=zs[:],
                                 func=mybir.ActivationFunctionType.Exp,
                                 scale=1.0)
            nc.scalar.activation(out=zs[:], in_=zs[:],
                                 func=mybir.ActivationFunctionType.Exp,
                                 scale=NEG_SLOPE)
            nc.vector.tensor_tensor(out=s_self[:], in0=e1s[:], in1=zs[:],
                                    op=mybir.AluOpType.max)
            nc.vector.tensor_add(s_self[:], s_self[:], epst[:])

            # ---- phase A: build the table ----
            WB = 8  # blocks per load / table-write batch
            for g0 in range(0, nblk_tbl, WB):
                gn = min(WB, nblk_tbl - g0)
                tstage = tspool.tile([P, WB, f_out + 4], bf16)
                xtb8 = xtpool.tile([P, WB, P], bf16, tag="xtb")
                nc.sync.dma_start(
                    out=xtb8[:, 0:gn, :],
                    in_=xTb[:, g0 * P:(g0 + gn) * P].rearrange(
                        "p (i q) -> p i q", q=P))
                for bi in range(gn):
                    ps = pspool.tile([P, f_out + 2], fp32, tag="psh")
                    nc.tensor.matmul(out=ps[:], lhsT=xtb8[:, bi, :].squeeze(),
                                     rhs=wextb_sb[:, 0:f_out + 2],
                                     start=True, stop=True)
                    if bi % 2 == 0:
                        nc.scalar.copy(out=tstage[:, bi, 0:f_out],
                                       in_=ps[:, 0:f_out])
                    else:
                        nc.vector.tensor_copy(out=tstage[:, bi, 0:f_out],
                                              in_=ps[:, 0:f_out])
                    nc.vector.tensor_copy(
                        out=tstage[:, bi, f_out:f_out + 4].bitcast(fp32),
                        in_=ps[:, f_out:f_out + 2])
                r0 = g0 * P
                r1 = r0 + gn * P
                if r1 <= lo_rows:
                    dst_ap = tbl_lo[r0:r1, 0:f_out + 4]
                else:
                    assert r0 >= lo_rows
                    dst_ap = tbl_hi[r0 - lo_rows:r1 - lo_rows, 0:f_out + 4]
                nc.sync.dma_start(
                    out=dst_ap.rearrange("(i p) w -> p i w", p=P),
                    in_=tstage[:, 0:gn, :])
                if g0 == 0:
                    # overwrite row 0 with the pad row as soon as possible
                    nc.sync.dma_start(out=tbl_lo[0:1, :], in_=padrow[:])
            if hi_rows > 0:
                ph = cfg["n_nodes"] + 1 - lo_rows
                nc.sync.dma_start(out=tbl_hi[ph:ph + 1, :], in_=padrow[:])

            # ---- phase B: gather + softmax + weighted sum ----
            call_q = 0
            for sci, sc in enumerate(scs):
                clo, chi, off_lo, len_lo, off_hi, len_hi = calls[sci]
                csc = clo + chi
                nb = len(sc)
                i0 = sc[0]
                g_t = gpool.tile([P, csc, P], bf16)
                if clo > 0:
                    nc.gpsimd.dma_gather(
                        out_ap=g_t[:, 0:clo, :], in_ap=tbl_lo[:],
                        idxs_ap=gidx_sb[:, off_lo:off_lo + len_lo],
                        num_idxs=clo * P, num_idxs_reg=clo * P,
                        elem_size=P, single_packet=False,
                        queue_num=call_q % 4)
                    call_q += 1
                if chi > 0:
                    nc.gpsimd.dma_gather(
                        out_ap=g_t[:, clo:csc, :], in_ap=tbl_hi[:],
                        idxs_ap=gidx_sb[:, off_hi:off_hi + len_hi],
                        num_idxs=chi * P, num_idxs_reg=chi * P,
                        elem_size=P, single_packet=False,
                        queue_num=call_q % 4)
                    call_q += 1

                # s = exp(lrelu(z)) = max(exp(z), exp(0.2 z)) per block-half,
                # with z = asrc[src] + adst[dst] folded into the ACT bias.
                z_t = scpool.tile([P, csc], fp32, tag="z")
                e1_t = scpool.tile([P, csc], fp32, tag="e1")
                s_t = scpool.tile([P, csc], bf16, tag="s")
                dn_t = scpool.tile([P, 2 * nb], fp32, tag="dn")
                asrcv = g_t[:, :, f_out:f_out + 2].bitcast(fp32)
                for bi, i in enumerate(sc):
                    for h0, h1 in (
                        (int(col_off_lo[i] - col_off_lo[i0]),
                         int(col_off_lo[i + 1] - col_off_lo[i0])),
                        (clo + int(col_off_hi[i] - col_off_hi[i0]),
                         clo + int(col_off_hi[i + 1] - col_off_hi[i0])),
                    ):
                        if h1 > h0:
                            nc.scalar.activation(
                                out=e1_t[:, h0:h1],
                                in_=asrcv[:, h0:h1].squeeze(),
                                func=mybir.ActivationFunctionType.Exp,
                                bias=adst_own[:, i:i + 1], scale=1.0)
                            nc.scalar.activation(
                                out=z_t[:, h0:h1],
                                in_=asrcv[:, h0:h1].squeeze(),
                                func=mybir.ActivationFunctionType.Exp,
                                bias=adst02[:, i:i + 1], scale=NEG_SLOPE)
                nc.vector.tensor_tensor(out=s_t[:], in0=e1_t[:], in1=z_t[:],
                                        op=mybir.AluOpType.max)
                for bi, i in enumerate(sc):
                    for half, (h0, h1) in enumerate((
                        (int(col_off_lo[i] - col_off_lo[i0]),
                         int(col_off_lo[i + 1] - col_off_lo[i0])),
                        (clo + int(col_off_hi[i] - col_off_hi[i0]),
                         clo + int(col_off_hi[i + 1] - col_off_hi[i0])),
                    )):
                        dslice = dn_t[:, 2 * bi + half:2 * bi + half + 1]
                        if h1 == h0:
                            nc.vector.memset(dslice, 0.0)
                        else:
                            nc.vector.tensor_reduce(
                                out=dslice, in_=s_t[:, h0:h1],
                                axis=mybir.AxisListType.X,
                                op=mybir.AluOpType.add)

                dsum = bpool.tile([P, nb], fp32, tag="dsum")
                nc.vector.tensor_reduce(
                    out=dsum[:],
                    in_=dn_t[:].rearrange("p (b t) -> p b t", t=2),
                    axis=mybir.AxisListType.X,
                    op=mybir.AluOpType.add)
                rec = bpool.tile([P, nb], fp32, tag="rec")
                nc.vector.tensor_add(dsum[:], dsum[:],
                                     s_self[:, i0:i0 + nb])
                nc.vector.reciprocal(rec[:], dsum[:])

                wgt = scpool.tile([P, csc, f_out], bf16, tag="wgt")
                nc.vector.tensor_tensor(
                    out=wgt[:], in0=g_t[:, :, 0:f_out],
                    in1=s_t[:].unsqueeze(2).broadcast_to([P, csc, f_out]),
                    op=mybir.AluOpType.mult)

                t1a = bpool.tile([P, nb, f_out], fp32, tag="t1a")
                t2a = bpool.tile([P, nb, f_out], fp32, tag="t2a")
                ostage = scpool.tile([P, nb, f_out], fp32, tag="ostage")
                for bi, i in enumerate(sc):
                    a0 = int(col_off_lo[i] - col_off_lo[i0])
                    a1 = int(col_off_lo[i + 1] - col_off_lo[i0])
                    b0 = clo + int(col_off_hi[i] - col_off_hi[i0])
                    b1 = clo + int(col_off_hi[i + 1] - col_off_hi[i0])
                    # self-loop contribution seeds t2a
                    nc.vector.tensor_scalar_mul(
                        t2a[:, bi, :], h_own[:, i, :], s_self[:, i:i + 1])
                    if a1 > a0:
                        nc.vector.tensor_reduce(
                            out=t1a[:, bi, :],
                            in_=wgt[:, a0:a1, :].rearrange("p c f -> p f c"),
                            axis=mybir.AxisListType.X, op=mybir.AluOpType.add)
                    else:
                        nc.vector.memset(t1a[:, bi, :], 0.0)
                    if b1 > b0:
                        tmp = bpool.tile([P, f_out], fp32, tag="tmp2")
                        nc.vector.tensor_reduce(
                            out=tmp[:],
                            in_=wgt[:, b0:b1, :].rearrange("p c f -> p f c"),
                            axis=mybir.AxisListType.X, op=mybir.AluOpType.add)
                        nc.vector.tensor_add(t2a[:, bi, :], t2a[:, bi, :],
                                             tmp[:])
                nc.vector.tensor_add(t1a[:], t1a[:], t2a[:])
                nc.vector.tensor_tensor(
                    out=t1a[:], in0=t1a[:],
                    in1=rec[:].unsqueeze(2).broadcast_to([P, nb, f_out]),
                    op=mybir.AluOpType.mult)
                nc.vector.tensor_tensor(
                    out=t1a[:], in0=t1a[:],
                    in1=biasb_sb[:].unsqueeze(1).broadcast_to([P, nb, f_out]),
                    op=mybir.AluOpType.add)
                nc.scalar.activation(out=ostage[:], in_=t1a[:],
                                     func=mybir.ActivationFunctionType.Relu)
                nc.sync.dma_start(
                    out=out_d[i0 * P:(i0 + nb) * P, :].rearrange(
                        "(i p) f -> p i f", p=P),
                    in_=ostage[:])
    nc.compile()
    return nc


def _gat_kernel(x, edge_index, W, att_src, att_dst, bias, lo_rows=32768,
                cmax=96):
    n_nodes, f_in = x.shape
    f_out = W.shape[1]
    assert f_in == P
    trows = _ceil_to(n_nodes + 2, P)
    lo_rows = min(lo_rows, trows)

    meta = _preprocess(edge_index, n_nodes, lo_rows)
    scs = _make_superchunks(meta["d_lo"], meta["d_hi"], cmax)
    gidx, calls, gc16 = _build_gidx(meta, scs)

    cfg = dict(trows=trows, lo_rows=lo_rows, slots=meta["slots"], scs=scs,
               calls=calls, d_lo=meta["d_lo"], d_hi=meta["d_hi"],
               col_off_lo=meta["col_off_lo"], col_off_hi=meta["col_off_hi"],
               gc16=gc16, f_out=f_out, n_nodes=n_nodes)
    nc = _build_nc(cfg)
    _LAST_META[0] = (meta, cfg)

    # ---- inputs ----
    x = np.asarray(x, dtype=np.float32)
    W = np.asarray(W, dtype=np.float32)
    att_src = np.asarray(att_src, dtype=np.float32)
    att_dst = np.asarray(att_dst, dtype=np.float32)
    bias = np.asarray(bias, dtype=np.float32)

    xT = np.zeros((P, trows), dtype=np.float32)
    xT[:, 1:1 + n_nodes] = x.T
    wext = np.zeros((P, f_out + 2), dtype=np.float32)
    wext[:, 0:f_out] = W
    wext[:, f_out] = W @ att_src
    wext[:, f_out + 1] = W @ att_dst
    xTb = xT.astype(ml_dtypes.bfloat16)
    wextb = wext.astype(ml_dtypes.bfloat16)
    biasb = np.tile(bias[None, :], (P, 1)).astype(np.float32)
    # table row = P bf16 cols: [h x f_out | a_src f32 as 2 cols | pad]
    padrow_f32 = np.zeros(P // 2, dtype=np.float32)
    padrow_f32[f_out // 2] = PAD_ASRC  # f32 word 32 == bf16 cols 64..65
    padrow = padrow_f32.view(ml_dtypes.bfloat16).reshape(1, P).copy()

    in_maps = []
    for k in range(NCORES):
        ox = np.zeros((P, meta["slots"] * P), dtype=np.float32)
        nd = meta["node_at"][k::NCORES].reshape(-1)  # blocks k, k+8,... -> slots
        m = nd >= 0
        ox[:, m] = x[nd[m]].T
        gi = gidx[k]
        if gi.shape[1] < max(gc16, 16):
            gi = np.concatenate(
                [gi, np.zeros((P, max(gc16, 16) - gi.shape[1]), np.int16)], axis=1)
        in_maps.append({
            "xTb": xTb, "wextb": wextb,
            "ownxt": ox.astype(ml_dtypes.bfloat16),
            "gidx": np.ascontiguousarray(gi),
            "biasb": biasb,
            "padrow": padrow,
        })

    res = run_bass_kernel_spmd(nc, in_maps, core_ids=list(range(NCORES)),
                               **_RUN_KW)
    _LAST_RESULT[0] = res

    out = np.zeros((n_nodes, f_out), dtype=np.float32)
    for k in range(NCORES):
        nd = meta["node_at"][k::NCORES].reshape(-1)
        m = nd >= 0
        out[nd[m]] = res.results[k]["out"][m]
    return out


_RUN_KW = {}
_LAST_RESULT = [None]
_LAST_META = [None]


def kernel(x, edge_index, W, att_src, att_dst, bias):
    return _gat_kernel(x, edge_index, W, att_src, att_dst, bias)
